# revision 11
# baseline (speedup 1.0000x reference)
"""Bahdanau attention TRN2 Bass kernel (8-core data-parallel over batch).

Problem: B=64, T=2048, D=U=256 (fp32)
  q_proj = query @ W1 + b1                     [B, U]
  v_proj = values @ W2 + b2                    [B, T, U]
  score  = tanh(q_proj + v_proj) @ V + Vb      [B, T, 1]
  attn   = softmax(score, axis=1)
  ctx    = sum(attn * values, axis=1)          [B, D]

Sharding: batch dim across 8 cores (8 batches/core); weights replicated.
V_b is a constant shift before softmax -> mathematically irrelevant, dropped.

Per-core dataflow (all fp32/f32r):
  - values loaded as natural tiles nat[128(T),...,256(D)] (resident, 128KB/par)
  - per (batch, T-512 chunk): PE-transpose -> valT [D,T] chunks; v_projT =
    W2.T @ valT (PSUM); ACT tanh evac with per-partition bias qprojT[:,bi]
    -> tanhT; score matvec with replicated-V lhsT [128,32] -> score rows at
    32-aligned PSUM partitions (4 batches per bank group)
  - scores compacted via SBUF->SBUF DMA (partition-strided reads are DMA-only)
  - single-pass softmax without max subtraction (scores provably bounded by
    ||V||_1 * 1 + |Vb| ~ 10): exp with accum_out row-sums, reciprocal, scale
  - exp'd scores PE-transposed back to T-partitioned columns eT [128,8] per
    T-128 chunk; context matvecs accumulate over T in PSUM; scaled evac
"""
import numpy as np

import concourse.bacc as bacc
import concourse.bass as bass
import concourse.tile as tile
import concourse.mybir as mybir
from concourse.bass_utils import run_bass_kernel_spmd
from concourse.masks import make_identity
from contextlib import ExitStack

F32 = mybir.dt.float32
F32R = mybir.dt.float32r
AF = mybir.ActivationFunctionType

B, T, D, U = 64, 2048, 256, 256
N_CORES = 8
BL = B // N_CORES          # 8 local batches
NT = T // 128              # 16 T-128 chunks
NTC = T // 512             # 4 T-512 chunks
DC = D // 128              # 2 contraction chunks
UC = U // 128              # 2 U chunks

_NC_CACHE = {}


def _build_module():
    nc = bacc.Bacc("TRN2", debug=False)

    q_h = nc.dram_tensor("query", [BL, D], F32R, kind="ExternalInput")
    v_h = nc.dram_tensor("values", [BL, T, D], F32R, kind="ExternalInput")
    w1_h = nc.dram_tensor("W1_w", [D, U], F32R, kind="ExternalInput")
    b1_h = nc.dram_tensor("W1_b", [U], F32R, kind="ExternalInput")
    w2_h = nc.dram_tensor("W2_w", [D, U], F32R, kind="ExternalInput")
    b2_h = nc.dram_tensor("W2_b", [U], F32R, kind="ExternalInput")
    vw_h = nc.dram_tensor("V_w", [U, 1], F32R, kind="ExternalInput")
    ctx_h = nc.dram_tensor("context", [BL, D], F32, kind="ExternalOutput")
    attn_h = nc.dram_tensor("attn", [BL, T], F32R, kind="ExternalOutput")

    query = q_h.ap()
    values = v_h.ap()

    with tile.TileContext(nc) as tc, ExitStack() as ctx:
        consts = ctx.enter_context(tc.tile_pool(name="consts", bufs=1))
        persist = ctx.enter_context(tc.tile_pool(name="persist", bufs=1))
        work = ctx.enter_context(tc.tile_pool(name="work", bufs=3))

        # ---------------- constants (ACT-ring + SWDGE DMAs) ----------------
        ident_f = consts.tile([128, 128], F32)
        make_identity(nc, ident_f)
        ident = consts.tile([128, 128], F32R)
        nc.vector.tensor_copy(ident[:], ident_f[:])

        w1_sb = consts.tile([128, DC, U], F32R)
        nc.scalar.dma_start(out=w1_sb[:], in_=w1_h.ap().rearrange("(c p) u -> p c u", p=128))
        w2_sb = consts.tile([128, DC, U], F32R)
        nc.scalar.dma_start(out=w2_sb[:], in_=w2_h.ap().rearrange("(c p) u -> p c u", p=128))

        # V weight columns [128, uc] and combined bias (b1+b2) [128, uc]
        vcol = consts.tile([128, UC], F32R)
        b1t = consts.tile([128, UC], F32R)
        b2t = consts.tile([128, UC], F32R)
        for c in range(UC):
            nc.gpsimd.dma_start(out=vcol[:, c:c + 1], in_=vw_h.ap()[c * 128:(c + 1) * 128, 0:1])
            nc.scalar.dma_start(out=b1t[:, c:c + 1],
                                in_=bass.AP(tensor=b1_h, offset=c * 128, ap=[[1, 128], [1, 1]]))
            nc.scalar.dma_start(out=b2t[:, c:c + 1],
                                in_=bass.AP(tensor=b2_h, offset=c * 128, ap=[[1, 128], [1, 1]]))
        cbt = consts.tile([128, UC], F32R)
        nc.vector.tensor_add(cbt[:], b1t[:], b2t[:])

        # ---------------- values natural tiles (SP-ring DMAs) ----------------
        nat = persist.tile([128, BL, NT, D], F32R)   # 128KB/partition, resident
        for bi in range(BL):
            nc.sync.dma_start(
                out=nat[:, bi, :, :],
                in_=values[bi].rearrange("(t p) d -> p t d", p=128),
            )

        # ---------------- qprojT = W1.T @ q.T + (b1+b2) ----------------
        q_sb = consts.tile([BL, D], F32R)
        nc.scalar.dma_start(out=q_sb[:], in_=query)
        qpt = persist.tile([128, UC, BL], F32R)
        with tc.tile_pool(name="pq", bufs=2, space="PSUM") as pq:
            qt_sb = consts.tile([128, DC, BL], F32R)
            for dc in range(DC):
                qt_ps = pq.tile([128, BL], F32R)
                nc.tensor.matmul(qt_ps[:], q_sb[:, dc * 128:(dc + 1) * 128],
                                 ident[0:BL, 0:BL], is_transpose=True)
                nc.vector.tensor_copy(qt_sb[:, dc, :], qt_ps[:])
            for uc in range(UC):
                qp_ps = pq.tile([128, BL], F32)
                for dc in range(DC):
                    nc.tensor.matmul(qp_ps[:], w1_sb[:, dc, uc * 128:(uc + 1) * 128],
                                     qt_sb[:, dc, :], start=(dc == 0), stop=(dc == DC - 1))
                nc.scalar.activation(qpt[:, uc, :], qp_ps[:], AF.Identity,
                                     bias=cbt[:, uc:uc + 1])

        # ---------------- main loop: scores ----------------
        s_cmp = persist.tile([BL, T], F32R)    # compact scores, later attn out
        with (
            tc.tile_pool(name="psc", bufs=4, space="PSUM") as psc_pool,
            tc.tile_pool(name="ptr", bufs=2, space="PSUM") as ptr_pool,
            tc.tile_pool(name="pvp", bufs=2, space="PSUM") as pvp_pool,
        ):
            for bi in range(BL):
                for tcn in range(NTC):
                    valt = work.tile([128, DC, 512], F32R, tag="valt")
                    for dc in range(DC):
                        ptr = ptr_pool.tile([128, 4, 128], F32R, tag="ptr")
                        for k in range(4):
                            nc.tensor.matmul(
                                ptr[:, k, :],
                                nat[:, bi, tcn * 4 + k, dc * 128:(dc + 1) * 128],
                                ident[:], is_transpose=True, skip_group_check=True)
                        nc.vector.tensor_copy(valt[:, dc, :], ptr[:])
                    psc = psc_pool.tile([1, 512], F32, tag="psc")
                    for uc in range(UC):
                        pvp = pvp_pool.tile([128, 512], F32, tag="pvp")
                        for dc in range(DC):
                            nc.tensor.matmul(
                                pvp[:], w2_sb[:, dc, uc * 128:(uc + 1) * 128],
                                valt[:, dc, :], start=(dc == 0), stop=(dc == DC - 1))
                        tanh_t = work.tile([128, 512], F32R, tag="tanh")
                        nc.scalar.activation(tanh_t[:], pvp[:], AF.Tanh,
                                             bias=qpt[:, uc, bi:bi + 1])
                        nc.tensor.matmul(psc[:], vcol[:, uc:uc + 1], tanh_t[:],
                                         start=(uc == 0), stop=(uc == UC - 1))
                    stage = work.tile([1, 512], F32R, tag="stage", bufs=6)
                    if (bi + tcn) % 2 == 0:
                        nc.vector.tensor_copy(stage[:], psc[:])
                    else:
                        nc.scalar.copy(stage[:], psc[:])
                    nc.sync.dma_start(out=s_cmp[bi:bi + 1, tcn * 512:(tcn + 1) * 512],
                                      in_=stage[:])

        # ---------------- softmax (no max-sub; scores bounded ~|V|_1) -------
        e_cmp = persist.tile([BL, T], F32R)
        sums = persist.tile([BL, 1], F32)
        nc.scalar.activation(e_cmp[:], s_cmp[:], AF.Exp, accum_out=sums[:])
        recip = persist.tile([BL, 1], F32)
        nc.vector.reciprocal(recip[:], sums[:])

        # normalized attention weights (reuse s_cmp tile), then DMA out
        nc.vector.tensor_scalar_mul(s_cmp[:], e_cmp[:], recip[:])
        nc.sync.dma_start(out=attn_h.ap(), in_=s_cmp[:])

        # ---------------- wT: normalized weights back to T-partitions -------
        et_sb = persist.tile([128, NT, BL], F32R)
        with tc.tile_pool(name="pwt", bufs=1, space="PSUM") as pwt_pool:
            pwt = pwt_pool.tile([128, NT, BL], F32R)
            for k in range(NT):
                nc.tensor.matmul(pwt[:, k, :], s_cmp[:, k * 128:(k + 1) * 128],
                                 ident[0:BL, 0:BL], is_transpose=True,
                                 skip_group_check=True)
            nc.vector.tensor_copy(et_sb[:], pwt[:])


        # ---------------- context = values.T @ w (normalized) ---------------
        with tc.tile_pool(name="pctx", bufs=4, space="PSUM") as pctx_pool:
            for bi in range(BL):
                pctx = pctx_pool.tile([1, D], F32, tag="pctx")
                for k in range(NT):
                    nc.tensor.matmul(pctx[:], et_sb[:, k, bi:bi + 1], nat[:, bi, k, :],
                                     start=(k == 0), stop=(k == NT - 1))
                cstage = work.tile([1, D], F32, tag="cstage", bufs=4)
                nc.vector.tensor_copy(cstage[:], pctx[:])
                nc.sync.dma_start(out=ctx_h.ap()[bi:bi + 1, :], in_=cstage[:])

    nc.compile()
    return nc


def _get_module():
    if "nc" not in _NC_CACHE:
        _NC_CACHE["nc"] = _build_module()
    return _NC_CACHE["nc"]


def _make_in_maps(inputs):
    q = np.ascontiguousarray(np.asarray(inputs["query"], dtype=np.float32))
    v = np.ascontiguousarray(np.asarray(inputs["values"], dtype=np.float32))
    shared = {
        "W1_w": np.ascontiguousarray(np.asarray(inputs["W1_w"], dtype=np.float32)),
        "W1_b": np.ascontiguousarray(np.asarray(inputs["W1_b"], dtype=np.float32)),
        "W2_w": np.ascontiguousarray(np.asarray(inputs["W2_w"], dtype=np.float32)),
        "W2_b": np.ascontiguousarray(np.asarray(inputs["W2_b"], dtype=np.float32)),
        "V_w": np.ascontiguousarray(np.asarray(inputs["V_w"], dtype=np.float32)),
    }
    in_maps = []
    for i in range(N_CORES):
        m = dict(shared)
        m["query"] = np.ascontiguousarray(q[i * BL:(i + 1) * BL])
        m["values"] = np.ascontiguousarray(v[i * BL:(i + 1) * BL])
        in_maps.append(m)
    return in_maps


def _assemble(results):
    ctx = np.concatenate([results[i]["context"] for i in range(N_CORES)], axis=0)
    attn = np.concatenate([results[i]["attn"] for i in range(N_CORES)], axis=0)
    return ctx.astype(np.float32), attn.reshape(B, T, 1).astype(np.float32)


def kernel(**inputs):
    nc = _get_module()
    in_maps = _make_in_maps(inputs)
    res = run_bass_kernel_spmd(nc, in_maps, list(range(N_CORES)))
    return _assemble(res.results)


if __name__ == "__main__":
    rng = np.random.default_rng(0)
    inputs = {
        "query": rng.standard_normal((B, D), dtype=np.float32),
        "values": rng.standard_normal((B, T, D), dtype=np.float32),
        "W1_w": (rng.standard_normal((D, U)) * 0.05).astype(np.float32),
        "W1_b": np.zeros(U, np.float32),
        "W2_w": (rng.standard_normal((D, U)) * 0.05).astype(np.float32),
        "W2_b": np.zeros(U, np.float32),
        "V_w": (rng.standard_normal((U, 1)) * 0.05).astype(np.float32),
        "V_b": np.zeros(1, np.float32),
    }
    ctx_out, attn_out = kernel(**inputs)
    # numpy reference
    qp = inputs["query"] @ inputs["W1_w"] + inputs["W1_b"]
    vp = np.einsum("btd,du->btu", inputs["values"], inputs["W2_w"]) + inputs["W2_b"]
    sc = np.tanh(qp[:, None, :] + vp) @ inputs["V_w"] + inputs["V_b"]
    sc = sc - sc.max(axis=1, keepdims=True)
    e = np.exp(sc)
    aw = e / e.sum(axis=1, keepdims=True)
    cv = (aw * inputs["values"]).sum(axis=1)
    print("ctx relerr:", np.abs(ctx_out - cv).max() / np.abs(cv).max())
    print("attn relerr:", np.abs(attn_out - aw).max() / np.abs(aw).max())


# revision 14
# speedup vs baseline: 188.1747x; 188.1747x over previous
"""Bahdanau attention TRN2 Bass kernel (8-core data-parallel over batch).

Problem: B=64, T=2048, D=U=256 (fp32)
  q_proj = query @ W1 + b1                     [B, U]
  v_proj = values @ W2 + b2                    [B, T, U]
  score  = tanh(q_proj + v_proj) @ V + Vb      [B, T, 1]
  attn   = softmax(score, axis=1)
  ctx    = sum(attn * values, axis=1)          [B, D]

Sharding: batch dim across 8 cores (8 batches/core); weights replicated.
V_b is a constant shift before softmax -> mathematically irrelevant, dropped.

Per-core dataflow (all fp32/f32r):
  - values loaded as natural tiles nat[128(T),...,256(D)] (resident, 128KB/par)
  - per (batch, T-512 chunk): PE-transpose -> valT [D,T] chunks; v_projT =
    W2.T @ valT (PSUM); ACT tanh evac with per-partition bias qprojT[:,bi]
    -> tanhT; score matvec with replicated-V lhsT [128,32] -> score rows at
    32-aligned PSUM partitions (4 batches per bank group)
  - scores compacted via SBUF->SBUF DMA (partition-strided reads are DMA-only)
  - single-pass softmax without max subtraction (scores provably bounded by
    ||V||_1 * 1 + |Vb| ~ 10): exp with accum_out row-sums, reciprocal, scale
  - exp'd scores PE-transposed back to T-partitioned columns eT [128,8] per
    T-128 chunk; context matvecs accumulate over T in PSUM; scaled evac
"""
import numpy as np

import concourse.bacc as bacc
import concourse.bass as bass
import concourse.tile as tile
import concourse.mybir as mybir
from concourse.bass_utils import run_bass_kernel_spmd
from concourse.masks import make_identity
from contextlib import ExitStack

F32 = mybir.dt.float32
F32R = mybir.dt.float32r
AF = mybir.ActivationFunctionType

B, T, D, U = 64, 2048, 256, 256
N_CORES = 8
BL = B // N_CORES          # 8 local batches
NT = T // 128              # 16 T-128 chunks
NTC = T // 512             # 4 T-512 chunks
DC = D // 128              # 2 contraction chunks
UC = U // 128              # 2 U chunks

_NC_CACHE = {}


def _build_module():
    nc = bacc.Bacc("TRN2", debug=False)

    q_h = nc.dram_tensor("query", [BL, D], F32R, kind="ExternalInput")
    v_h = nc.dram_tensor("values", [BL, T, D], F32R, kind="ExternalInput")
    w1_h = nc.dram_tensor("W1_w", [D, U], F32R, kind="ExternalInput")
    b1_h = nc.dram_tensor("W1_b", [U], F32R, kind="ExternalInput")
    w2_h = nc.dram_tensor("W2_w", [D, U], F32R, kind="ExternalInput")
    b2_h = nc.dram_tensor("W2_b", [U], F32R, kind="ExternalInput")
    vw_h = nc.dram_tensor("V_w", [U, 1], F32R, kind="ExternalInput")
    ctx_h = nc.dram_tensor("context", [BL, D], F32, kind="ExternalOutput")
    attn_h = nc.dram_tensor("attn", [BL, T], F32R, kind="ExternalOutput")

    query = q_h.ap()
    values = v_h.ap()

    with tile.TileContext(nc) as tc, ExitStack() as ctx:
        consts = ctx.enter_context(tc.tile_pool(name="consts", bufs=1))
        persist = ctx.enter_context(tc.tile_pool(name="persist", bufs=1))
        work = ctx.enter_context(tc.tile_pool(name="work", bufs=3))

        # ---------------- constants (ACT-ring + SWDGE DMAs) ----------------
        ident_f = consts.tile([128, 128], F32)
        make_identity(nc, ident_f)
        ident = consts.tile([128, 128], F32R)
        nc.vector.tensor_copy(ident[:], ident_f[:])

        w1_sb = consts.tile([128, DC, U], F32R)
        nc.scalar.dma_start(out=w1_sb[:], in_=w1_h.ap().rearrange("(c p) u -> p c u", p=128))
        w2_sb = consts.tile([128, DC, U], F32R)
        nc.scalar.dma_start(out=w2_sb[:], in_=w2_h.ap().rearrange("(c p) u -> p c u", p=128))

        # V weight columns [128, uc] and combined bias (b1+b2) [128, uc]
        vcol = consts.tile([128, UC], F32R)
        b1t = consts.tile([128, UC], F32R)
        b2t = consts.tile([128, UC], F32R)
        for c in range(UC):
            nc.gpsimd.dma_start(out=vcol[:, c:c + 1], in_=vw_h.ap()[c * 128:(c + 1) * 128, 0:1])
            nc.scalar.dma_start(out=b1t[:, c:c + 1],
                                in_=bass.AP(tensor=b1_h, offset=c * 128, ap=[[1, 128], [1, 1]]))
            nc.scalar.dma_start(out=b2t[:, c:c + 1],
                                in_=bass.AP(tensor=b2_h, offset=c * 128, ap=[[1, 128], [1, 1]]))
        cbt = consts.tile([128, UC], F32R)
        nc.vector.tensor_add(cbt[:], b1t[:], b2t[:])

        # ---------------- values natural tiles (SP-ring DMAs) ----------------
        nat = persist.tile([128, BL, NT, D], F32R)   # 128KB/partition, resident
        for bi in range(BL):
            nc.sync.dma_start(
                out=nat[:, bi, :, :],
                in_=values[bi].rearrange("(t p) d -> p t d", p=128),
            )

        # ---------------- qprojT = W1.T @ q.T + (b1+b2) ----------------
        q_sb = consts.tile([BL, D], F32R)
        nc.scalar.dma_start(out=q_sb[:], in_=query)
        qpt = persist.tile([128, UC, BL], F32R)
        with tc.tile_pool(name="pq", bufs=2, space="PSUM") as pq:
            qt_sb = consts.tile([128, DC, BL], F32R)
            for dc in range(DC):
                qt_ps = pq.tile([128, BL], F32R)
                nc.tensor.matmul(qt_ps[:], q_sb[:, dc * 128:(dc + 1) * 128],
                                 ident[0:BL, 0:BL], is_transpose=True)
                nc.vector.tensor_copy(qt_sb[:, dc, :], qt_ps[:])
            for uc in range(UC):
                qp_ps = pq.tile([128, BL], F32)
                for dc in range(DC):
                    nc.tensor.matmul(qp_ps[:], w1_sb[:, dc, uc * 128:(uc + 1) * 128],
                                     qt_sb[:, dc, :], start=(dc == 0), stop=(dc == DC - 1))
                nc.scalar.activation(qpt[:, uc, :], qp_ps[:], AF.Identity,
                                     bias=cbt[:, uc:uc + 1])

        # ---------------- main loop: scores ----------------
        s_cmp = persist.tile([BL, T], F32R)    # compact scores, later attn out
        with (
            tc.tile_pool(name="psc", bufs=4, space="PSUM") as psc_pool,
            tc.tile_pool(name="ptr", bufs=2, space="PSUM") as ptr_pool,
            tc.tile_pool(name="pvp", bufs=2, space="PSUM") as pvp_pool,
        ):
            for bi in range(BL):
                for tcn in range(NTC):
                    valt = work.tile([128, DC, 512], F32R, tag="valt")
                    for dc in range(DC):
                        ptr = ptr_pool.tile([128, 4, 128], F32R, tag="ptr")
                        for k in range(4):
                            nc.tensor.matmul(
                                ptr[:, k, :],
                                nat[:, bi, tcn * 4 + k, dc * 128:(dc + 1) * 128],
                                ident[:], is_transpose=True, skip_group_check=True)
                        nc.vector.tensor_copy(valt[:, dc, :], ptr[:])
                    psc = psc_pool.tile([1, 512], F32, tag="psc")
                    for uc in range(UC):
                        pvp = pvp_pool.tile([128, 512], F32, tag="pvp")
                        for dc in range(DC):
                            nc.tensor.matmul(
                                pvp[:], w2_sb[:, dc, uc * 128:(uc + 1) * 128],
                                valt[:, dc, :], start=(dc == 0), stop=(dc == DC - 1))
                        tanh_t = work.tile([128, 512], F32R, tag="tanh")
                        nc.scalar.activation(tanh_t[:], pvp[:], AF.Tanh,
                                             bias=qpt[:, uc, bi:bi + 1])
                        nc.tensor.matmul(psc[:], vcol[:, uc:uc + 1], tanh_t[:],
                                         start=(uc == 0), stop=(uc == UC - 1))
                    stage = work.tile([1, 512], F32R, tag="stage", bufs=6)
                    if (bi + tcn) % 2 == 0:
                        nc.vector.tensor_copy(stage[:], psc[:])
                    else:
                        nc.scalar.copy(stage[:], psc[:])
                    nc.sync.dma_start(out=s_cmp[bi:bi + 1, tcn * 512:(tcn + 1) * 512],
                                      in_=stage[:])

        # ---------------- softmax (no max-sub; scores bounded ~|V|_1) -------
        e_cmp = persist.tile([BL, T], F32R)
        sums = persist.tile([BL, 1], F32)
        nc.scalar.activation(e_cmp[:], s_cmp[:], AF.Exp, accum_out=sums[:])
        recip = persist.tile([BL, 1], F32)
        nc.vector.reciprocal(recip[:], sums[:])

        # normalized attention weights (reuse s_cmp tile), then DMA out
        nc.vector.tensor_scalar_mul(s_cmp[:], e_cmp[:], recip[:])
        nc.sync.dma_start(out=attn_h.ap(), in_=s_cmp[:])

        # ---------------- wT: normalized weights back to T-partitions -------
        et_sb = persist.tile([128, NT, BL], F32R)
        with tc.tile_pool(name="pwt", bufs=1, space="PSUM") as pwt_pool:
            pwt = pwt_pool.tile([128, NT, BL], F32R)
            for k in range(NT):
                nc.tensor.matmul(pwt[:, k, :], s_cmp[:, k * 128:(k + 1) * 128],
                                 ident[0:BL, 0:BL], is_transpose=True,
                                 skip_group_check=True)
            nc.vector.tensor_copy(et_sb[:], pwt[:])


        # ---------------- context = values.T @ w (normalized) ---------------
        with tc.tile_pool(name="pctx", bufs=4, space="PSUM") as pctx_pool:
            for bi in range(BL):
                pctx = pctx_pool.tile([1, D], F32, tag="pctx")
                for k in range(NT):
                    nc.tensor.matmul(pctx[:], et_sb[:, k, bi:bi + 1], nat[:, bi, k, :],
                                     start=(k == 0), stop=(k == NT - 1))
                cstage = work.tile([1, D], F32, tag="cstage", bufs=4)
                nc.vector.tensor_copy(cstage[:], pctx[:])
                nc.sync.dma_start(out=ctx_h.ap()[bi:bi + 1, :], in_=cstage[:])

    nc.compile()
    return nc


def _get_module():
    if "nc" not in _NC_CACHE:
        _NC_CACHE["nc"] = _build_module()
    return _NC_CACHE["nc"]


def _make_in_maps(inputs):
    q = np.ascontiguousarray(np.asarray(inputs["query"], dtype=np.float32))
    v = np.ascontiguousarray(np.asarray(inputs["values"], dtype=np.float32))
    shared = {
        "W1_w": np.ascontiguousarray(np.asarray(inputs["W1_w"], dtype=np.float32)),
        "W1_b": np.ascontiguousarray(np.asarray(inputs["W1_b"], dtype=np.float32)),
        "W2_w": np.ascontiguousarray(np.asarray(inputs["W2_w"], dtype=np.float32)),
        "W2_b": np.ascontiguousarray(np.asarray(inputs["W2_b"], dtype=np.float32)),
        "V_w": np.ascontiguousarray(np.asarray(inputs["V_w"], dtype=np.float32)),
    }
    in_maps = []
    for i in range(N_CORES):
        m = dict(shared)
        m["query"] = np.ascontiguousarray(q[i * BL:(i + 1) * BL])
        m["values"] = np.ascontiguousarray(v[i * BL:(i + 1) * BL])
        in_maps.append(m)
    return in_maps


def _assemble(results):
    ctx = np.concatenate([results[i]["context"] for i in range(N_CORES)], axis=0)
    attn = np.concatenate([results[i]["attn"] for i in range(N_CORES)], axis=0)
    return ctx.astype(np.float32), attn.reshape(B, T, 1).astype(np.float32)


def _make_runner(nc):
    """Cached jit runner replicating bass2jax.run_bass_via_pjrt's multi-core
    path, but building the sharded executable exactly once."""
    import jax
    from jax.experimental.shard_map import shard_map
    from jax.sharding import Mesh, PartitionSpec
    from concourse import bass2jax

    bass2jax.install_neuronx_cc_hook()
    assert nc.dbg_addr is None
    partition_name = nc.partition_id_tensor.name if nc.partition_id_tensor else None
    in_names, out_names, out_avals, zero_outs = [], [], [], []
    for alloc in nc.m.functions[0].allocations:
        if not isinstance(alloc, mybir.MemoryLocationSet):
            continue
        name = alloc.memorylocations[0].name
        if alloc.kind == "ExternalInput":
            if name != partition_name:
                in_names.append(name)
        elif alloc.kind == "ExternalOutput":
            out_names.append(name)
            shape = tuple(alloc.tensor_shape)
            dtype = mybir.dt.np(alloc.dtype)
            out_avals.append(jax.core.ShapedArray(shape, dtype))
            zero_outs.append(np.zeros(shape, dtype))
    n_params, n_outs = len(in_names), len(out_avals)
    in_names_full = list(in_names) + out_names
    if partition_name is not None:
        in_names_full.append(partition_name)
    donate = tuple(range(n_params, n_params + n_outs))

    def _body(*args):
        operands = list(args)
        if partition_name is not None:
            operands.append(bass2jax.partition_id_tensor())
        outs = bass2jax._bass_exec_p.bind(
            *operands, out_avals=tuple(out_avals), in_names=tuple(in_names_full),
            out_names=tuple(out_names), lowering_input_output_aliases=(),
            sim_require_finite=True, sim_require_nnan=True, nc=nc)
        return tuple(outs)

    devices = jax.devices()[:N_CORES]
    mesh = Mesh(np.asarray(devices), ("core",))
    sharded = jax.jit(
        shard_map(_body, mesh=mesh,
                  in_specs=(PartitionSpec("core"),) * (n_params + n_outs),
                  out_specs=(PartitionSpec("core"),) * n_outs, check_rep=False),
        donate_argnums=donate, keep_unused=True)

    def pack(in_maps):
        per_core = [[np.asarray(m[name]) for name in in_names] for m in in_maps]
        return [np.concatenate([per_core[c][i] for c in range(N_CORES)], axis=0)
                for i in range(n_params)]

    def make_zeros():
        return [np.zeros((N_CORES * z.shape[0], *z.shape[1:]), z.dtype)
                for z in zero_outs]

    def run(concat_in):
        out_arrs = sharded(*concat_in, *make_zeros())
        return [{name: np.asarray(out_arrs[i]).reshape(N_CORES, *out_avals[i].shape)[c]
                 for i, name in enumerate(out_names)} for c in range(N_CORES)]

    run.pack = pack
    run.make_zeros = make_zeros
    run.sharded = sharded
    run.mesh = mesh
    return run


def _get_runner():
    if "runner" not in _NC_CACHE:
        _NC_CACHE["runner"] = _make_runner(_get_module())
    return _NC_CACHE["runner"]


def kernel(**inputs):
    runner = _get_runner()
    results = runner(runner.pack(_make_in_maps(inputs)))
    return _assemble(results)


if __name__ == "__main__":
    rng = np.random.default_rng(0)
    inputs = {
        "query": rng.standard_normal((B, D), dtype=np.float32),
        "values": rng.standard_normal((B, T, D), dtype=np.float32),
        "W1_w": (rng.standard_normal((D, U)) * 0.05).astype(np.float32),
        "W1_b": np.zeros(U, np.float32),
        "W2_w": (rng.standard_normal((D, U)) * 0.05).astype(np.float32),
        "W2_b": np.zeros(U, np.float32),
        "V_w": (rng.standard_normal((U, 1)) * 0.05).astype(np.float32),
        "V_b": np.zeros(1, np.float32),
    }
    ctx_out, attn_out = kernel(**inputs)
    # numpy reference
    qp = inputs["query"] @ inputs["W1_w"] + inputs["W1_b"]
    vp = np.einsum("btd,du->btu", inputs["values"], inputs["W2_w"]) + inputs["W2_b"]
    sc = np.tanh(qp[:, None, :] + vp) @ inputs["V_w"] + inputs["V_b"]
    sc = sc - sc.max(axis=1, keepdims=True)
    e = np.exp(sc)
    aw = e / e.sum(axis=1, keepdims=True)
    cv = (aw * inputs["values"]).sum(axis=1)
    print("ctx relerr:", np.abs(ctx_out - cv).max() / np.abs(cv).max())
    print("attn relerr:", np.abs(attn_out - aw).max() / np.abs(aw).max())


# revision 15
# speedup vs baseline: 1550.3581x; 8.2389x over previous
"""Bahdanau attention TRN2 Bass kernel (8-core data-parallel over batch).

Problem: B=64, T=2048, D=U=256 (fp32)
  q_proj = query @ W1 + b1                     [B, U]
  v_proj = values @ W2 + b2                    [B, T, U]
  score  = tanh(q_proj + v_proj) @ V + Vb      [B, T, 1]
  attn   = softmax(score, axis=1)
  ctx    = sum(attn * values, axis=1)          [B, D]

Sharding: batch dim across 8 cores (8 batches/core); weights replicated.
V_b is a constant shift before softmax -> mathematically irrelevant, dropped.

Per-core dataflow (all fp32/f32r):
  - values loaded as natural tiles nat[128(T),...,256(D)] (resident, 128KB/par)
  - per (batch, T-512 chunk): PE-transpose -> valT [D,T] chunks; v_projT =
    W2.T @ valT (PSUM); ACT tanh evac with per-partition bias qprojT[:,bi]
    -> tanhT; score matvec with replicated-V lhsT [128,32] -> score rows at
    32-aligned PSUM partitions (4 batches per bank group)
  - scores compacted via SBUF->SBUF DMA (partition-strided reads are DMA-only)
  - single-pass softmax without max subtraction (scores provably bounded by
    ||V||_1 * 1 + |Vb| ~ 10): exp with accum_out row-sums, reciprocal, scale
  - exp'd scores PE-transposed back to T-partitioned columns eT [128,8] per
    T-128 chunk; context matvecs accumulate over T in PSUM; scaled evac
"""
import numpy as np

import concourse.bacc as bacc
import concourse.bass as bass
import concourse.tile as tile
import concourse.mybir as mybir
from concourse.bass_utils import run_bass_kernel_spmd
from concourse.masks import make_identity
from contextlib import ExitStack

F32 = mybir.dt.float32
F32R = mybir.dt.float32r
AF = mybir.ActivationFunctionType

B, T, D, U = 64, 2048, 256, 256
N_CORES = 8
BL = B // N_CORES          # 8 local batches
NT = T // 128              # 16 T-128 chunks
NTC = T // 512             # 4 T-512 chunks
DC = D // 128              # 2 contraction chunks
UC = U // 128              # 2 U chunks

_NC_CACHE = {}


def _build_module(reps=1):
    nc = bacc.Bacc("TRN2", debug=False)

    q_h = nc.dram_tensor("query", [BL, D], F32R, kind="ExternalInput")
    v_h = nc.dram_tensor("values", [BL, T, D], F32R, kind="ExternalInput")
    w1_h = nc.dram_tensor("W1_w", [D, U], F32R, kind="ExternalInput")
    b1_h = nc.dram_tensor("W1_b", [U], F32R, kind="ExternalInput")
    w2_h = nc.dram_tensor("W2_w", [D, U], F32R, kind="ExternalInput")
    b2_h = nc.dram_tensor("W2_b", [U], F32R, kind="ExternalInput")
    vw_h = nc.dram_tensor("V_w", [U, 1], F32R, kind="ExternalInput")
    ctx_h = nc.dram_tensor("context", [BL, D], F32, kind="ExternalOutput")
    attn_h = nc.dram_tensor("attn", [BL, T], F32R, kind="ExternalOutput")

    query = q_h.ap()
    values = v_h.ap()

    with tile.TileContext(nc) as tc, ExitStack() as ctx:
        consts = ctx.enter_context(tc.tile_pool(name="consts", bufs=1))
        persist = ctx.enter_context(tc.tile_pool(name="persist", bufs=1))
        work = ctx.enter_context(tc.tile_pool(name="work", bufs=3))

        # ---------------- constants (ACT-ring + SWDGE DMAs) ----------------
        ident_f = consts.tile([128, 128], F32)
        make_identity(nc, ident_f)
        ident = consts.tile([128, 128], F32R)
        nc.vector.tensor_copy(ident[:], ident_f[:])

        w1_sb = consts.tile([128, DC, U], F32R)
        nc.scalar.dma_start(out=w1_sb[:], in_=w1_h.ap().rearrange("(c p) u -> p c u", p=128))
        w2_sb = consts.tile([128, DC, U], F32R)
        nc.scalar.dma_start(out=w2_sb[:], in_=w2_h.ap().rearrange("(c p) u -> p c u", p=128))

        # V weight columns [128, uc] and combined bias (b1+b2) [128, uc]
        vcol = consts.tile([128, UC], F32R)
        b1t = consts.tile([128, UC], F32R)
        b2t = consts.tile([128, UC], F32R)
        for c in range(UC):
            nc.gpsimd.dma_start(out=vcol[:, c:c + 1], in_=vw_h.ap()[c * 128:(c + 1) * 128, 0:1])
            nc.scalar.dma_start(out=b1t[:, c:c + 1],
                                in_=bass.AP(tensor=b1_h, offset=c * 128, ap=[[1, 128], [1, 1]]))
            nc.scalar.dma_start(out=b2t[:, c:c + 1],
                                in_=bass.AP(tensor=b2_h, offset=c * 128, ap=[[1, 128], [1, 1]]))
        cbt = consts.tile([128, UC], F32R)
        nc.vector.tensor_add(cbt[:], b1t[:], b2t[:])

        # ---------------- qprojT = W1.T @ q.T + (b1+b2) ----------------
        q_sb = consts.tile([BL, D], F32R)
        nc.scalar.dma_start(out=q_sb[:], in_=query)
        qpt = persist.tile([128, UC, BL], F32R)
        with tc.tile_pool(name="pq", bufs=2, space="PSUM") as pq:
            qt_sb = consts.tile([128, DC, BL], F32R)
            for dc in range(DC):
                qt_ps = pq.tile([128, BL], F32R)
                nc.tensor.matmul(qt_ps[:], q_sb[:, dc * 128:(dc + 1) * 128],
                                 ident[0:BL, 0:BL], is_transpose=True)
                nc.vector.tensor_copy(qt_sb[:, dc, :], qt_ps[:])
            for uc in range(UC):
                qp_ps = pq.tile([128, BL], F32)
                for dc in range(DC):
                    nc.tensor.matmul(qp_ps[:], w1_sb[:, dc, uc * 128:(uc + 1) * 128],
                                     qt_sb[:, dc, :], start=(dc == 0), stop=(dc == DC - 1))
                nc.scalar.activation(qpt[:, uc, :], qp_ps[:], AF.Identity,
                                     bias=cbt[:, uc:uc + 1])

        for rep in range(reps):
          # -------------- values natural tiles (SP-ring DMAs) ---------------
          nat = persist.tile([128, BL, NT, D], F32R, name="nat", tag="nat")
          for bi in range(BL):
            nc.sync.dma_start(
                out=nat[:, bi, :, :],
                in_=values[bi].rearrange("(t p) d -> p t d", p=128),
            )
          # ---------------- main loop: scores ----------------
          s_cmp = persist.tile([BL, T], F32R, name="s_cmp", tag="s_cmp")
          with (
            tc.tile_pool(name=f"psc{rep}", bufs=4, space="PSUM") as psc_pool,
            tc.tile_pool(name=f"ptr{rep}", bufs=2, space="PSUM") as ptr_pool,
            tc.tile_pool(name=f"pvp{rep}", bufs=2, space="PSUM") as pvp_pool,
          ):
            for bi in range(BL):
                for tcn in range(NTC):
                    valt = work.tile([128, DC, 512], F32R, name="valt", tag="valt")
                    for dc in range(DC):
                        ptr = ptr_pool.tile([128, 4, 128], F32R, name="ptr", tag="ptr")
                        for k in range(4):
                            nc.tensor.matmul(
                                ptr[:, k, :],
                                nat[:, bi, tcn * 4 + k, dc * 128:(dc + 1) * 128],
                                ident[:], is_transpose=True, skip_group_check=True)
                        nc.vector.tensor_copy(valt[:, dc, :], ptr[:])
                    psc = psc_pool.tile([1, 512], F32, name="psc", tag="psc")
                    for uc in range(UC):
                        pvp = pvp_pool.tile([128, 512], F32, name="pvp", tag="pvp")
                        for dc in range(DC):
                            nc.tensor.matmul(
                                pvp[:], w2_sb[:, dc, uc * 128:(uc + 1) * 128],
                                valt[:, dc, :], start=(dc == 0), stop=(dc == DC - 1))
                        tanh_t = work.tile([128, 512], F32R, name="tanh_t", tag="tanh")
                        nc.scalar.activation(tanh_t[:], pvp[:], AF.Tanh,
                                             bias=qpt[:, uc, bi:bi + 1])
                        nc.tensor.matmul(psc[:], vcol[:, uc:uc + 1], tanh_t[:],
                                         start=(uc == 0), stop=(uc == UC - 1))
                    stage = work.tile([1, 512], F32R, name="stage", tag="stage", bufs=6)
                    if (bi + tcn) % 2 == 0:
                        nc.vector.tensor_copy(stage[:], psc[:])
                    else:
                        nc.scalar.copy(stage[:], psc[:])
                    nc.sync.dma_start(out=s_cmp[bi:bi + 1, tcn * 512:(tcn + 1) * 512],
                                      in_=stage[:])

          # ---------------- softmax (no max-sub; scores bounded ~|V|_1) -----
          e_cmp = persist.tile([BL, T], F32R, name="e_cmp", tag="e_cmp")
          sums = persist.tile([BL, 1], F32, name="sums", tag="sums")
          nc.scalar.activation(e_cmp[:], s_cmp[:], AF.Exp, accum_out=sums[:])
          recip = persist.tile([BL, 1], F32, name="recip", tag="recip")
          nc.vector.reciprocal(recip[:], sums[:])

          # normalized attention weights (reuse s_cmp tile), then DMA out
          nc.vector.tensor_scalar_mul(s_cmp[:], e_cmp[:], recip[:])
          nc.sync.dma_start(out=attn_h.ap(), in_=s_cmp[:])

          # ---------------- wT: normalized weights back to T-partitions -----
          et_sb = persist.tile([128, NT, BL], F32R, name="et_sb", tag="et_sb")
          with tc.tile_pool(name=f"pwt{rep}", bufs=1, space="PSUM") as pwt_pool:
            pwt = pwt_pool.tile([128, NT, BL], F32R, name="pwt", tag="pwt")
            for k in range(NT):
                nc.tensor.matmul(pwt[:, k, :], s_cmp[:, k * 128:(k + 1) * 128],
                                 ident[0:BL, 0:BL], is_transpose=True,
                                 skip_group_check=True)
            nc.vector.tensor_copy(et_sb[:], pwt[:])

          # ---------------- context = values.T @ w (normalized) -------------
          with tc.tile_pool(name=f"pctx{rep}", bufs=4, space="PSUM") as pctx_pool:
            for bi in range(BL):
                pctx = pctx_pool.tile([1, D], F32, name="pctx", tag="pctx")
                for k in range(NT):
                    nc.tensor.matmul(pctx[:], et_sb[:, k, bi:bi + 1], nat[:, bi, k, :],
                                     start=(k == 0), stop=(k == NT - 1))
                cstage = work.tile([1, D], F32, name="cstage", tag="cstage", bufs=4)
                nc.vector.tensor_copy(cstage[:], pctx[:])
                nc.sync.dma_start(out=ctx_h.ap()[bi:bi + 1, :], in_=cstage[:])

    nc.compile()
    return nc


def _get_module(reps=1):
    key = f"nc{reps}"
    if key not in _NC_CACHE:
        _NC_CACHE[key] = _build_module(reps)
    return _NC_CACHE[key]


def _make_in_maps(inputs):
    q = np.ascontiguousarray(np.asarray(inputs["query"], dtype=np.float32))
    v = np.ascontiguousarray(np.asarray(inputs["values"], dtype=np.float32))
    shared = {
        "W1_w": np.ascontiguousarray(np.asarray(inputs["W1_w"], dtype=np.float32)),
        "W1_b": np.ascontiguousarray(np.asarray(inputs["W1_b"], dtype=np.float32)),
        "W2_w": np.ascontiguousarray(np.asarray(inputs["W2_w"], dtype=np.float32)),
        "W2_b": np.ascontiguousarray(np.asarray(inputs["W2_b"], dtype=np.float32)),
        "V_w": np.ascontiguousarray(np.asarray(inputs["V_w"], dtype=np.float32)),
    }
    in_maps = []
    for i in range(N_CORES):
        m = dict(shared)
        m["query"] = np.ascontiguousarray(q[i * BL:(i + 1) * BL])
        m["values"] = np.ascontiguousarray(v[i * BL:(i + 1) * BL])
        in_maps.append(m)
    return in_maps


def _assemble(results):
    ctx = np.concatenate([results[i]["context"] for i in range(N_CORES)], axis=0)
    attn = np.concatenate([results[i]["attn"] for i in range(N_CORES)], axis=0)
    return ctx.astype(np.float32), attn.reshape(B, T, 1).astype(np.float32)


def _make_runner(nc):
    """Cached jit runner replicating bass2jax.run_bass_via_pjrt's multi-core
    path, but building the sharded executable exactly once."""
    import jax
    from jax.experimental.shard_map import shard_map
    from jax.sharding import Mesh, PartitionSpec
    from concourse import bass2jax

    bass2jax.install_neuronx_cc_hook()
    assert nc.dbg_addr is None
    partition_name = nc.partition_id_tensor.name if nc.partition_id_tensor else None
    in_names, out_names, out_avals, zero_outs = [], [], [], []
    for alloc in nc.m.functions[0].allocations:
        if not isinstance(alloc, mybir.MemoryLocationSet):
            continue
        name = alloc.memorylocations[0].name
        if alloc.kind == "ExternalInput":
            if name != partition_name:
                in_names.append(name)
        elif alloc.kind == "ExternalOutput":
            out_names.append(name)
            shape = tuple(alloc.tensor_shape)
            dtype = mybir.dt.np(alloc.dtype)
            out_avals.append(jax.core.ShapedArray(shape, dtype))
            zero_outs.append(np.zeros(shape, dtype))
    n_params, n_outs = len(in_names), len(out_avals)
    in_names_full = list(in_names) + out_names
    if partition_name is not None:
        in_names_full.append(partition_name)
    donate = tuple(range(n_params, n_params + n_outs))

    def _body(*args):
        operands = list(args)
        if partition_name is not None:
            operands.append(bass2jax.partition_id_tensor())
        outs = bass2jax._bass_exec_p.bind(
            *operands, out_avals=tuple(out_avals), in_names=tuple(in_names_full),
            out_names=tuple(out_names), lowering_input_output_aliases=(),
            sim_require_finite=True, sim_require_nnan=True, nc=nc)
        return tuple(outs)

    devices = jax.devices()[:N_CORES]
    mesh = Mesh(np.asarray(devices), ("core",))
    sharded = jax.jit(
        shard_map(_body, mesh=mesh,
                  in_specs=(PartitionSpec("core"),) * (n_params + n_outs),
                  out_specs=(PartitionSpec("core"),) * n_outs, check_rep=False),
        donate_argnums=donate, keep_unused=True)

    def pack(in_maps):
        per_core = [[np.asarray(m[name]) for name in in_names] for m in in_maps]
        return [np.concatenate([per_core[c][i] for c in range(N_CORES)], axis=0)
                for i in range(n_params)]

    def make_zeros():
        return [np.zeros((N_CORES * z.shape[0], *z.shape[1:]), z.dtype)
                for z in zero_outs]

    def run(concat_in):
        out_arrs = sharded(*concat_in, *make_zeros())
        return [{name: np.asarray(out_arrs[i]).reshape(N_CORES, *out_avals[i].shape)[c]
                 for i, name in enumerate(out_names)} for c in range(N_CORES)]

    run.pack = pack
    run.make_zeros = make_zeros
    run.sharded = sharded
    run.mesh = mesh
    return run


def _get_runner(reps=1):
    key = f"runner{reps}"
    if key not in _NC_CACHE:
        _NC_CACHE[key] = _make_runner(_get_module(reps))
    return _NC_CACHE[key]


def kernel(**inputs):
    runner = _get_runner()
    results = runner(runner.pack(_make_in_maps(inputs)))
    return _assemble(results)


if __name__ == "__main__":
    rng = np.random.default_rng(0)
    inputs = {
        "query": rng.standard_normal((B, D), dtype=np.float32),
        "values": rng.standard_normal((B, T, D), dtype=np.float32),
        "W1_w": (rng.standard_normal((D, U)) * 0.05).astype(np.float32),
        "W1_b": np.zeros(U, np.float32),
        "W2_w": (rng.standard_normal((D, U)) * 0.05).astype(np.float32),
        "W2_b": np.zeros(U, np.float32),
        "V_w": (rng.standard_normal((U, 1)) * 0.05).astype(np.float32),
        "V_b": np.zeros(1, np.float32),
    }
    ctx_out, attn_out = kernel(**inputs)
    # numpy reference
    qp = inputs["query"] @ inputs["W1_w"] + inputs["W1_b"]
    vp = np.einsum("btd,du->btu", inputs["values"], inputs["W2_w"]) + inputs["W2_b"]
    sc = np.tanh(qp[:, None, :] + vp) @ inputs["V_w"] + inputs["V_b"]
    sc = sc - sc.max(axis=1, keepdims=True)
    e = np.exp(sc)
    aw = e / e.sum(axis=1, keepdims=True)
    cv = (aw * inputs["values"]).sum(axis=1)
    print("ctx relerr:", np.abs(ctx_out - cv).max() / np.abs(cv).max())
    print("attn relerr:", np.abs(attn_out - aw).max() / np.abs(aw).max())


# revision 16
# speedup vs baseline: 5819.1812x; 3.7534x over previous
"""Bahdanau attention TRN2 Bass kernel (8-core data-parallel over batch).

Problem: B=64, T=2048, D=U=256 (fp32)
  q_proj = query @ W1 + b1                     [B, U]
  v_proj = values @ W2 + b2                    [B, T, U]
  score  = tanh(q_proj + v_proj) @ V + Vb      [B, T, 1]
  attn   = softmax(score, axis=1)
  ctx    = sum(attn * values, axis=1)          [B, D]

Sharding: batch dim across 8 cores (8 batches/core); weights replicated.
V_b is a constant shift before softmax -> mathematically irrelevant, dropped.

Per-core dataflow (all fp32/f32r):
  - values loaded as natural tiles nat[128(T),...,256(D)] (resident, 128KB/par)
  - per (batch, T-512 chunk): PE-transpose -> valT [D,T] chunks; v_projT =
    W2.T @ valT (PSUM); ACT tanh evac with per-partition bias qprojT[:,bi]
    -> tanhT; score matvec with replicated-V lhsT [128,32] -> score rows at
    32-aligned PSUM partitions (4 batches per bank group)
  - scores compacted via SBUF->SBUF DMA (partition-strided reads are DMA-only)
  - single-pass softmax without max subtraction (scores provably bounded by
    ||V||_1 * 1 + |Vb| ~ 10): exp with accum_out row-sums, reciprocal, scale
  - exp'd scores PE-transposed back to T-partitioned columns eT [128,8] per
    T-128 chunk; context matvecs accumulate over T in PSUM; scaled evac
"""
import numpy as np

import concourse.bacc as bacc
import concourse.bass as bass
import concourse.tile as tile
import concourse.mybir as mybir
from concourse.bass_utils import run_bass_kernel_spmd
from concourse.masks import make_identity
from contextlib import ExitStack

F32 = mybir.dt.float32
F32R = mybir.dt.float32r
AF = mybir.ActivationFunctionType

B, T, D, U = 64, 2048, 256, 256
N_CORES = 8
BL = B // N_CORES          # 8 local batches
NT = T // 128              # 16 T-128 chunks
NTC = T // 512             # 4 T-512 chunks
DC = D // 128              # 2 contraction chunks
UC = U // 128              # 2 U chunks

_NC_CACHE = {}


def _build_module(reps=1):
    nc = bacc.Bacc("TRN2", debug=False)

    q_h = nc.dram_tensor("query", [BL, D], F32R, kind="ExternalInput")
    v_h = nc.dram_tensor("values", [BL, T, D], F32R, kind="ExternalInput")
    w1_h = nc.dram_tensor("W1_w", [D, U], F32R, kind="ExternalInput")
    b1_h = nc.dram_tensor("W1_b", [U], F32R, kind="ExternalInput")
    w2_h = nc.dram_tensor("W2_w", [D, U], F32R, kind="ExternalInput")
    b2_h = nc.dram_tensor("W2_b", [U], F32R, kind="ExternalInput")
    vw_h = nc.dram_tensor("V_w", [U, 1], F32R, kind="ExternalInput")
    ctx_h = nc.dram_tensor("context", [BL, D], F32, kind="ExternalOutput")
    attn_h = nc.dram_tensor("attn", [BL, T], F32R, kind="ExternalOutput")

    query = q_h.ap()
    values = v_h.ap()

    with tile.TileContext(nc) as tc, ExitStack() as ctx:
        consts = ctx.enter_context(tc.tile_pool(name="consts", bufs=1))
        persist = ctx.enter_context(tc.tile_pool(name="persist", bufs=1))
        work = ctx.enter_context(tc.tile_pool(name="work", bufs=3))

        # ---------------- constants (ACT-ring + SWDGE DMAs) ----------------
        ident_f = consts.tile([128, 128], F32)
        make_identity(nc, ident_f)
        ident = consts.tile([128, 128], F32R)
        nc.vector.tensor_copy(ident[:], ident_f[:])

        w1_sb = consts.tile([128, DC, U], F32R)
        nc.scalar.dma_start(out=w1_sb[:], in_=w1_h.ap().rearrange("(c p) u -> p c u", p=128))
        w2_sb = consts.tile([128, DC, U], F32R)
        nc.scalar.dma_start(out=w2_sb[:], in_=w2_h.ap().rearrange("(c p) u -> p c u", p=128))

        # V weight columns [128, uc] and combined bias (b1+b2) [128, uc]
        vcol = consts.tile([128, UC], F32R)
        b1t = consts.tile([128, UC], F32R)
        b2t = consts.tile([128, UC], F32R)
        for c in range(UC):
            nc.gpsimd.dma_start(out=vcol[:, c:c + 1], in_=vw_h.ap()[c * 128:(c + 1) * 128, 0:1])
            nc.scalar.dma_start(out=b1t[:, c:c + 1],
                                in_=bass.AP(tensor=b1_h, offset=c * 128, ap=[[1, 128], [1, 1]]))
            nc.scalar.dma_start(out=b2t[:, c:c + 1],
                                in_=bass.AP(tensor=b2_h, offset=c * 128, ap=[[1, 128], [1, 1]]))
        cbt = consts.tile([128, UC], F32R)
        nc.vector.tensor_add(cbt[:], b1t[:], b2t[:])

        # ---------------- qprojT = W1.T @ q.T + (b1+b2) ----------------
        q_sb = consts.tile([BL, D], F32R)
        nc.scalar.dma_start(out=q_sb[:], in_=query)
        qpt = persist.tile([128, UC, BL], F32R)
        with tc.tile_pool(name="pq", bufs=2, space="PSUM") as pq:
            qt_sb = consts.tile([128, DC, BL], F32R)
            for dc in range(DC):
                qt_ps = pq.tile([128, BL], F32R)
                nc.tensor.matmul(qt_ps[:], q_sb[:, dc * 128:(dc + 1) * 128],
                                 ident[0:BL, 0:BL], is_transpose=True)
                nc.vector.tensor_copy(qt_sb[:, dc, :], qt_ps[:])
            for uc in range(UC):
                qp_ps = pq.tile([128, BL], F32)
                for dc in range(DC):
                    nc.tensor.matmul(qp_ps[:], w1_sb[:, dc, uc * 128:(uc + 1) * 128],
                                     qt_sb[:, dc, :], start=(dc == 0), stop=(dc == DC - 1))
                nc.scalar.activation(qpt[:, uc, :], qp_ps[:], AF.Identity,
                                     bias=cbt[:, uc:uc + 1])

        for rep in range(reps):
          # -------------- values natural tiles (SP-ring DMAs) ---------------
          nat = persist.tile([128, BL, NT, D], F32R, name="nat", tag="nat")
          for bi in range(BL):
            nc.sync.dma_start(
                out=nat[:, bi, :, :],
                in_=values[bi].rearrange("(t p) d -> p t d", p=128),
            )
          # ---------------- main loop: scores ----------------
          s_cmp = persist.tile([BL, T], F32R, name="s_cmp", tag="s_cmp")
          with (
            tc.tile_pool(name=f"psc{rep}", bufs=2, space="PSUM") as psc_pool,
            tc.tile_pool(name=f"ptr{rep}", bufs=3, space="PSUM") as ptr_pool,
            tc.tile_pool(name=f"pvp{rep}", bufs=3, space="PSUM") as pvp_pool,
          ):
            for bi in range(BL):
                for tcn in range(NTC):
                    valt = work.tile([128, DC, 512], F32R, name="valt", tag="valt")
                    for dc in range(DC):
                        ptr = ptr_pool.tile([128, 4, 128], F32R, name="ptr", tag="ptr")
                        for k in range(4):
                            nc.tensor.matmul(
                                ptr[:, k, :],
                                nat[:, bi, tcn * 4 + k, dc * 128:(dc + 1) * 128],
                                ident[:], is_transpose=True, skip_group_check=True)
                        nc.vector.tensor_copy(valt[:, dc, :], ptr[:])
                    psc = psc_pool.tile([1, 512], F32, name="psc", tag="psc")
                    for uc in range(UC):
                        pvp = pvp_pool.tile([128, 512], F32, name="pvp", tag="pvp")
                        for dc in range(DC):
                            nc.tensor.matmul(
                                pvp[:], w2_sb[:, dc, uc * 128:(uc + 1) * 128],
                                valt[:, dc, :], start=(dc == 0), stop=(dc == DC - 1))
                        tanh_t = work.tile([128, 512], F32R, name="tanh_t", tag="tanh")
                        nc.scalar.activation(tanh_t[:], pvp[:], AF.Tanh,
                                             bias=qpt[:, uc, bi:bi + 1])
                        nc.tensor.matmul(psc[:], vcol[:, uc:uc + 1], tanh_t[:],
                                         start=(uc == 0), stop=(uc == UC - 1))
                    stage = work.tile([1, 512], F32R, name="stage", tag="stage", bufs=6)
                    if (bi + tcn) % 2 == 0:
                        nc.vector.tensor_copy(stage[:], psc[:])
                    else:
                        nc.scalar.copy(stage[:], psc[:])
                    nc.sync.dma_start(out=s_cmp[bi:bi + 1, tcn * 512:(tcn + 1) * 512],
                                      in_=stage[:])

          # ---------------- softmax (no max-sub; scores bounded ~|V|_1) -----
          e_cmp = persist.tile([BL, T], F32R, name="e_cmp", tag="e_cmp")
          sums = persist.tile([BL, 1], F32, name="sums", tag="sums")
          nc.scalar.activation(e_cmp[:], s_cmp[:], AF.Exp, accum_out=sums[:])
          recip = persist.tile([BL, 1], F32, name="recip", tag="recip")
          nc.vector.reciprocal(recip[:], sums[:])

          # normalized attention weights (reuse s_cmp tile), then DMA out
          nc.vector.tensor_scalar_mul(s_cmp[:], e_cmp[:], recip[:])
          nc.sync.dma_start(out=attn_h.ap(), in_=s_cmp[:])

          # ---------------- wT: normalized weights back to T-partitions -----
          et_sb = persist.tile([128, NT, BL], F32R, name="et_sb", tag="et_sb")
          with tc.tile_pool(name=f"pwt{rep}", bufs=1, space="PSUM") as pwt_pool:
            pwt = pwt_pool.tile([128, NT, BL], F32R, name="pwt", tag="pwt")
            for k in range(NT):
                nc.tensor.matmul(pwt[:, k, :], s_cmp[:, k * 128:(k + 1) * 128],
                                 ident[0:BL, 0:BL], is_transpose=True,
                                 skip_group_check=True)
            nc.vector.tensor_copy(et_sb[:], pwt[:])

          # ---------------- context = values.T @ w (normalized) -------------
          with tc.tile_pool(name=f"pctx{rep}", bufs=4, space="PSUM") as pctx_pool:
            for bi in range(BL):
                pctx = pctx_pool.tile([1, D], F32, name="pctx", tag="pctx")
                for k in range(NT):
                    nc.tensor.matmul(pctx[:], et_sb[:, k, bi:bi + 1], nat[:, bi, k, :],
                                     start=(k == 0), stop=(k == NT - 1))
                cstage = work.tile([1, D], F32, name="cstage", tag="cstage", bufs=4)
                nc.vector.tensor_copy(cstage[:], pctx[:])
                nc.sync.dma_start(out=ctx_h.ap()[bi:bi + 1, :], in_=cstage[:])

    nc.compile()
    return nc


def _get_module(reps=1):
    key = f"nc{reps}"
    if key not in _NC_CACHE:
        _NC_CACHE[key] = _build_module(reps)
    return _NC_CACHE[key]


def _make_in_maps(inputs):
    q = np.ascontiguousarray(np.asarray(inputs["query"], dtype=np.float32))
    v = np.ascontiguousarray(np.asarray(inputs["values"], dtype=np.float32))
    shared = {
        "W1_w": np.ascontiguousarray(np.asarray(inputs["W1_w"], dtype=np.float32)),
        "W1_b": np.ascontiguousarray(np.asarray(inputs["W1_b"], dtype=np.float32)),
        "W2_w": np.ascontiguousarray(np.asarray(inputs["W2_w"], dtype=np.float32)),
        "W2_b": np.ascontiguousarray(np.asarray(inputs["W2_b"], dtype=np.float32)),
        "V_w": np.ascontiguousarray(np.asarray(inputs["V_w"], dtype=np.float32)),
    }
    in_maps = []
    for i in range(N_CORES):
        m = dict(shared)
        m["query"] = np.ascontiguousarray(q[i * BL:(i + 1) * BL])
        m["values"] = np.ascontiguousarray(v[i * BL:(i + 1) * BL])
        in_maps.append(m)
    return in_maps


def _assemble(results):
    ctx = np.concatenate([results[i]["context"] for i in range(N_CORES)], axis=0)
    attn = np.concatenate([results[i]["attn"] for i in range(N_CORES)], axis=0)
    return ctx.astype(np.float32), attn.reshape(B, T, 1).astype(np.float32)


def _make_runner(nc):
    """Cached jit runner replicating bass2jax.run_bass_via_pjrt's multi-core
    path, but building the sharded executable exactly once."""
    import jax
    from jax.experimental.shard_map import shard_map
    from jax.sharding import Mesh, PartitionSpec
    from concourse import bass2jax

    bass2jax.install_neuronx_cc_hook()
    assert nc.dbg_addr is None
    partition_name = nc.partition_id_tensor.name if nc.partition_id_tensor else None
    in_names, out_names, out_avals, zero_outs = [], [], [], []
    for alloc in nc.m.functions[0].allocations:
        if not isinstance(alloc, mybir.MemoryLocationSet):
            continue
        name = alloc.memorylocations[0].name
        if alloc.kind == "ExternalInput":
            if name != partition_name:
                in_names.append(name)
        elif alloc.kind == "ExternalOutput":
            out_names.append(name)
            shape = tuple(alloc.tensor_shape)
            dtype = mybir.dt.np(alloc.dtype)
            out_avals.append(jax.core.ShapedArray(shape, dtype))
            zero_outs.append(np.zeros(shape, dtype))
    n_params, n_outs = len(in_names), len(out_avals)
    in_names_full = list(in_names) + out_names
    if partition_name is not None:
        in_names_full.append(partition_name)
    donate = tuple(range(n_params, n_params + n_outs))

    def _body(*args):
        operands = list(args)
        if partition_name is not None:
            operands.append(bass2jax.partition_id_tensor())
        outs = bass2jax._bass_exec_p.bind(
            *operands, out_avals=tuple(out_avals), in_names=tuple(in_names_full),
            out_names=tuple(out_names), lowering_input_output_aliases=(),
            sim_require_finite=True, sim_require_nnan=True, nc=nc)
        return tuple(outs)

    devices = jax.devices()[:N_CORES]
    mesh = Mesh(np.asarray(devices), ("core",))
    sharded = jax.jit(
        shard_map(_body, mesh=mesh,
                  in_specs=(PartitionSpec("core"),) * (n_params + n_outs),
                  out_specs=(PartitionSpec("core"),) * n_outs, check_rep=False),
        donate_argnums=donate, keep_unused=True)

    def pack(in_maps):
        per_core = [[np.asarray(m[name]) for name in in_names] for m in in_maps]
        return [np.concatenate([per_core[c][i] for c in range(N_CORES)], axis=0)
                for i in range(n_params)]

    def make_zeros():
        return [np.zeros((N_CORES * z.shape[0], *z.shape[1:]), z.dtype)
                for z in zero_outs]

    def run(concat_in):
        out_arrs = sharded(*concat_in, *make_zeros())
        return [{name: np.asarray(out_arrs[i]).reshape(N_CORES, *out_avals[i].shape)[c]
                 for i, name in enumerate(out_names)} for c in range(N_CORES)]

    run.pack = pack
    run.make_zeros = make_zeros
    run.sharded = sharded
    run.mesh = mesh
    return run


def _get_runner(reps=1):
    key = f"runner{reps}"
    if key not in _NC_CACHE:
        _NC_CACHE[key] = _make_runner(_get_module(reps))
    return _NC_CACHE[key]


def kernel(**inputs):
    runner = _get_runner()
    results = runner(runner.pack(_make_in_maps(inputs)))
    return _assemble(results)


if __name__ == "__main__":
    rng = np.random.default_rng(0)
    inputs = {
        "query": rng.standard_normal((B, D), dtype=np.float32),
        "values": rng.standard_normal((B, T, D), dtype=np.float32),
        "W1_w": (rng.standard_normal((D, U)) * 0.05).astype(np.float32),
        "W1_b": np.zeros(U, np.float32),
        "W2_w": (rng.standard_normal((D, U)) * 0.05).astype(np.float32),
        "W2_b": np.zeros(U, np.float32),
        "V_w": (rng.standard_normal((U, 1)) * 0.05).astype(np.float32),
        "V_b": np.zeros(1, np.float32),
    }
    ctx_out, attn_out = kernel(**inputs)
    # numpy reference
    qp = inputs["query"] @ inputs["W1_w"] + inputs["W1_b"]
    vp = np.einsum("btd,du->btu", inputs["values"], inputs["W2_w"]) + inputs["W2_b"]
    sc = np.tanh(qp[:, None, :] + vp) @ inputs["V_w"] + inputs["V_b"]
    sc = sc - sc.max(axis=1, keepdims=True)
    e = np.exp(sc)
    aw = e / e.sum(axis=1, keepdims=True)
    cv = (aw * inputs["values"]).sum(axis=1)
    print("ctx relerr:", np.abs(ctx_out - cv).max() / np.abs(cv).max())
    print("attn relerr:", np.abs(attn_out - aw).max() / np.abs(aw).max())
